# revision 14
# baseline (speedup 1.0000x reference)
"""GCNConv on 8 Trainium2 NeuronCores (Bass/Tile, SPMD).

out = selu((X @ W.T) * skip_w  +  segsum_dst(w_e * X[src] @ W.T)  + bias)

Key algebraic trick: the linear transform commutes with the (linear) edge
aggregation, so raw features are aggregated first and W is applied once per
destination block:   agg_dst = (sum_e w_e * X[src_e]) @ W.T
No h = X@W.T round-trip through DRAM; the kernel gathers raw 256B feature
rows with the custom SWDGE dma_gather (InstDMAGatherAnt, int16 indices,
single_packet=False — the only gather path that is correct on HW for
many-rows-per-partition gathers).

Sharding: destination nodes are degree-sorted and dealt round-robin across
(core, partition); band k = 1024 consecutive degree-ranked nodes = one
128-row block per core, padded to S[k] = max degree in band (~3% padding).
Since dma_gather indices are int16, bands are split into a few phases, each
with a per-(core,phase) compacted feature table (< 32768 rows) built on the
host (structural gather of input rows only).

Per group of bands: one dma_gather (~10k rows) -> msg [128, sumS*64]; one
DVE broadcast multiply by edge weights; per band a strided DVE reduce ->
aggX, concat with gathered x_dst, one PE transpose -> L = [aggX.T ; xd.T],
one K=128 matmul against [W.T ; W.T*skip] into a PSUM stage; batched SELU
epilogue (Relu/Exp on ACT) and one output DMA per group. Host work is
structural only (sorting, packing, dedup, permutation, replication).
"""
import sys

sys.path.insert(0, "/opt/trn_rl_repo")
import numpy as np

N = 50000
E = 800000
D = 64
P = 128
NC = 8
BAND = P * NC  # 1024 nodes per band
SELU_LAMBDA = 1.0507009873554804934193349852946
SELU_ALPHA = 1.6732632423543772848170429916717
SMAX = 96      # max slots/partition per gather group (NIDX <= 12288, HW-validated)
GMAX = 8       # max bands per group (PSUM bank: 8*64 f32 = one bank)
TAB_LIMIT = 30000  # per-phase compact table safety bound (int16 indices)

_compiled = {}


def _structure(edge_dst):
    deg = np.bincount(edge_dst, minlength=N)
    order = np.argsort(-deg, kind="stable").astype(np.int64)  # rank -> node
    rank = np.empty(N, dtype=np.int64)
    rank[order] = np.arange(N)
    nbands = (N + BAND - 1) // BAND  # 49
    degs_sorted = deg[order]
    S = np.zeros(nbands, dtype=np.int64)
    band_edges = np.zeros(nbands, dtype=np.int64)
    for k in range(nbands):
        d = degs_sorted[k * BAND : (k + 1) * BAND]
        S[k] = max(1, int(d.max()) if len(d) else 1)
        band_edges[k] = int(d.sum())
    # split bands into phases with ~equal edge counts; grow count until the
    # expected per-(core,phase) distinct-source bound is comfortable
    for nph in range(2, 9):
        tgt = band_edges.sum() / nph
        cuts = [0]
        acc = 0
        for k in range(nbands):
            acc += band_edges[k]
            if acc >= tgt * len(cuts) and len(cuts) < nph:
                cuts.append(k + 1)
        if cuts[-1] != nbands:
            cuts.append(nbands)
        phases = [(cuts[i], cuts[i + 1]) for i in range(len(cuts) - 1)]
        worst = max(band_edges[a:b].sum() for a, b in phases) / NC
        # distinct sources upper estimate + dst nodes
        est = N * (1.0 - np.exp(-worst / N)) + 1024 * max(b - a for a, b in phases) / NC
        if est < TAB_LIMIT - 3000:
            break
    # groups within phases
    groups = []  # (b0, g, ssum, phase_id)
    for pi, (a, b) in enumerate(phases):
        b0 = a
        while b0 < b:
            g = 0
            ssum = 0
            while b0 + g < b and g < GMAX and (g == 0 or ssum + S[b0 + g] <= SMAX):
                ssum += S[b0 + g]
                g += 1
            groups.append((b0, g, int(ssum), pi))
            b0 += g
    return deg, order, rank, S, phases, groups


def _rep16(flat):
    """int16 flat index list (i = s*128+p order) -> [128, len/16] replicated tile."""
    b16 = flat.reshape(-1, 16).T  # [16, n/16]
    return np.tile(b16, (8, 1))   # [128, n/16]


def _pack_host(features, edge_src, edge_dst, edge_weight, order, rank, S, phases, groups):
    nbands = len(S)
    nph = len(phases)
    band_phase = np.zeros(nbands, dtype=np.int64)
    for pi, (a, b) in enumerate(phases):
        band_phase[a:b] = pi

    # edge -> (core, band, partition, slot)
    sort_idx = np.argsort(edge_dst, kind="stable")
    sdst = edge_dst[sort_idx]
    ssrc = edge_src[sort_idx]
    swgt = edge_weight[sort_idx].astype(np.float32)
    first = np.searchsorted(sdst, sdst, side="left")
    pos = np.arange(len(sdst)) - first
    rr = rank[sdst]
    kb = rr // BAND
    cc = (rr // P) % NC
    pp = rr % P
    ph = band_phase[kb]

    # per-(core,phase) id tables
    ids_list = [[None] * nph for _ in range(NC)]
    tabsz = 0
    for c in range(NC):
        for pi in range(nph):
            a, b = phases[pi]
            m = (cc == c) & (ph == pi)
            dsts = []
            for k in range(a, b):
                rrs = k * BAND + c * P + np.arange(P)
                dsts.append(order[np.minimum(rrs, N - 1)])
            ids = np.unique(np.concatenate([ssrc[m]] + dsts))
            assert len(ids) < 32768, f"phase table overflow: {len(ids)}"
            ids_list[c][pi] = ids
            tabsz = max(tabsz, len(ids))
    tabsz = (tabsz + 15) // 16 * 16

    xtab = np.zeros((NC, nph * tabsz, D), dtype=np.float32)
    for c in range(NC):
        for pi in range(nph):
            ids = ids_list[c][pi]
            xtab[c, pi * tabsz : pi * tabsz + len(ids)] = features[ids]

    # slot packing:
    # per group: slot arrays A[c][(p, scol)] -> compact idx / weight
    band_g = np.zeros(nbands, dtype=np.int64)      # group id of band
    band_coloff = np.zeros(nbands, dtype=np.int64)
    for gi, (b0, g, ssum, pi) in enumerate(groups):
        c0 = 0
        for k in range(b0, b0 + g):
            band_g[k] = gi
            band_coloff[k] = c0
            c0 += S[k]

    idx16_chunks = [[] for _ in range(NC)]
    w_chunks = [[] for _ in range(NC)]
    idx16_off = []   # per-group offset into the flat idx16 array (elements)
    w_off = []       # per-group offset into w array
    io_acc = 0
    wo_acc = 0
    for gi, (b0, g, ssum, pi) in enumerate(groups):
        idx16_off.append(io_acc)
        w_off.append(wo_acc)
        io_acc += P * 8 * ssum  # replicated [128, 8*ssum]
        wo_acc += P * ssum

    # build per-core compact idx (int16) and weights arrays group by group
    scol = band_coloff[kb] + pos  # slot column within group
    for c in range(NC):
        mc = cc == c
        gsel = band_g[kb[mc]]
        A_idx = [np.zeros((P, ssum), dtype=np.int16) for (_, _, ssum, _) in groups]
        A_w = [np.zeros((P, ssum), dtype=np.float32) for (_, _, ssum, _) in groups]
        # remap sources to compact per-phase ids
        for pi in range(nph):
            mm = mc & (ph == pi)
            comp = np.searchsorted(ids_list[c][pi], ssrc[mm]).astype(np.int16)
            gsel2 = band_g[kb[mm]]
            p2 = pp[mm]
            s2 = scol[mm]
            # scatter into per-group arrays
            for gi in np.unique(gsel2):
                m3 = gsel2 == gi
                A_idx[gi][p2[m3], s2[m3]] = comp[m3]
                A_w[gi][p2[m3], s2[m3]] = swgt[mm][m3]
        for gi, (b0, g, ssum, pi) in enumerate(groups):
            flat = A_idx[gi].T.ravel()  # i = s*128+p order
            idx16_chunks[c].append(_rep16(flat).ravel())
            w_chunks[c].append(A_w[gi].ravel())

    idx16_all = np.stack([np.concatenate(ch) for ch in idx16_chunks])
    w_all = np.stack([np.concatenate(ch) for ch in w_chunks])

    # dst-node gathers per phase (xd), compact int16: flat i = b_local*128+p
    xd_chunks = [[] for _ in range(NC)]
    xd_off = []
    xo_acc = 0
    for pi, (a, b) in enumerate(phases):
        xd_off.append(xo_acc)
        xo_acc += P * 8 * (b - a)
    for c in range(NC):
        for pi, (a, b) in enumerate(phases):
            nb_ph = b - a
            Axd = np.zeros((P, nb_ph), dtype=np.int16)
            for bl, k in enumerate(range(a, b)):
                rrs = k * BAND + c * P + np.arange(P)
                nodes = order[np.minimum(rrs, N - 1)]
                comp = np.searchsorted(ids_list[c][pi], nodes)
                # pad nodes (rr >= N) may not be in the table; clip (discarded)
                comp = np.clip(comp, 0, len(ids_list[c][pi]) - 1)
                Axd[:, bl] = comp.astype(np.int16)
            flat = Axd.T.ravel()
            xd_chunks[c].append(_rep16(flat).ravel())
    idxd16_all = np.stack([np.concatenate(ch) for ch in xd_chunks])

    return (
        xtab,
        idx16_all,
        w_all,
        idxd16_all,
        tabsz,
        idx16_off,
        w_off,
        xd_off,
    )


def _build_program(S, phases, groups, tabsz, krep=1):
    import concourse.bass as bass
    import concourse.bacc as bacc
    import concourse.mybir as mybir
    import concourse.tile as tile
    from concourse.library_config import mlp

    nbands = len(S)
    nph = len(phases)
    fp = mybir.dt.float32
    i16 = mybir.dt.int16

    toti16 = sum(P * 8 * ssum for (_, _, ssum, _) in groups)
    totw = sum(P * ssum for (_, _, ssum, _) in groups)
    totd16 = sum(P * 8 * (b - a) for (a, b) in phases)

    nc = bacc.Bacc(None, target_bir_lowering=False, debug=False)
    xtab = nc.declare_dram_parameter("xtab", [nph * tabsz, D], fp, isOutput=False)
    idxp = nc.declare_dram_parameter("idx16", [toti16], i16, isOutput=False)
    wp_d = nc.declare_dram_parameter("wgt", [totw], fp, isOutput=False)
    idxd = nc.declare_dram_parameter("idxd16", [totd16], i16, isOutput=False)
    wtp = nc.declare_dram_parameter("wt", [P, D], fp, isOutput=False)      # [W.T; W.T]
    skp = nc.declare_dram_parameter("skr", [P, D], fp, isOutput=False)     # [ones; skip]
    bsp = nc.declare_dram_parameter("bsr", [P, D], fp, isOutput=False)
    idp = nc.declare_dram_parameter("ident", [P, P], fp, isOutput=False)
    outp = nc.declare_dram_parameter("out", [nbands * P, D], fp, isOutput=True)

    lam = SELU_LAMBDA
    la = SELU_LAMBDA * SELU_ALPHA

    with tile.TileContext(nc) as tc:
        with (
            tc.tile_pool(name="const", bufs=1) as cst,
            tc.tile_pool(name="msgp", bufs=2) as msgp,
            tc.tile_pool(name="meta", bufs=3) as meta,
            tc.tile_pool(name="xdp", bufs=2) as xdp,
            tc.tile_pool(name="lp", bufs=3) as lp,
            tc.tile_pool(name="stg", bufs=2) as stg,
            tc.tile_pool(name="psL", bufs=2, space="PSUM") as psL,
            tc.tile_pool(name="psZ", bufs=2, space="PSUM") as psZ,
        ):
            nc.gpsimd.load_library(mlp)
            ident = cst.tile([P, P], fp)
            nc.sync.dma_start(out=ident[:], in_=idp[:])
            wt_t = cst.tile([P, D], fp)
            nc.sync.dma_start(out=wt_t[:], in_=wtp[:])
            skr_t = cst.tile([P, D], fp)
            nc.sync.dma_start(out=skr_t[:], in_=skp[:])
            nc.vector.tensor_tensor(out=wt_t[:], in0=wt_t[:], in1=skr_t[:], op=mybir.AluOpType.mult)
            bsr_t = cst.tile([P, D], fp)
            nc.sync.dma_start(out=bsr_t[:], in_=bsp[:])

            for _rep in range(krep):
                for pi, (a, b) in enumerate(phases):
                    nb_ph = b - a
                    tab_ap = xtab[pi * tabsz : (pi + 1) * tabsz, :]
                    # dst features for this phase's bands
                    xd_off_ap = sum(P * 8 * (bb - aa) for (aa, bb) in phases[:pi])
                    itd = meta.tile([P, 8 * nb_ph], i16, tag="itd")
                    nc.sync.dma_start(
                        out=itd[:],
                        in_=idxd[xd_off_ap : xd_off_ap + P * 8 * nb_ph].rearrange(
                            "(p s) -> p s", p=P
                        ),
                    )
                    xd_ph = xdp.tile([P, nb_ph * D], fp, tag="xd")
                    nc.gpsimd.dma_gather(
                        xd_ph[:].rearrange("p (s c) -> p s c", c=D),
                        tab_ap,
                        itd[:, :],
                        P * nb_ph,
                        P * nb_ph,
                        D,
                        single_packet=False,
                    )
                    for gi, (b0, g, ssum, gpi) in enumerate(groups):
                        if gpi != pi:
                            continue
                        ioff = sum(P * 8 * s2 for (_, _, s2, _) in groups[:gi])
                        woff = sum(P * s2 for (_, _, s2, _) in groups[:gi])
                        it = meta.tile([P, 8 * ssum], i16, tag="it")
                        nc.sync.dma_start(
                            out=it[:],
                            in_=idxp[ioff : ioff + P * 8 * ssum].rearrange("(p s) -> p s", p=P),
                        )
                        wt_w = meta.tile([P, ssum], fp, tag="w")
                        nc.sync.dma_start(
                            out=wt_w[:],
                            in_=wp_d[woff : woff + P * ssum].rearrange("(p s) -> p s", p=P),
                        )
                        msg = msgp.tile([P, ssum * D], fp, tag="msg")
                        nc.gpsimd.dma_gather(
                            msg[:].rearrange("p (s c) -> p s c", c=D),
                            tab_ap,
                            it[:, :],
                            P * ssum,
                            P * ssum,
                            D,
                            single_packet=False,
                        )
                        # weight multiply, whole group in one DVE op
                        m3 = msg[:].rearrange("p (s c) -> p s c", c=D)
                        nc.vector.tensor_tensor(
                            out=m3, in0=m3,
                            in1=wt_w[:].unsqueeze(2).broadcast_to([P, ssum, D]),
                            op=mybir.AluOpType.mult,
                        )
                        zp = psZ.tile([P, g * D], fp, tag="z")
                        col = 0
                        for bi in range(g):
                            kband = b0 + bi
                            bloc = kband - a  # band index within phase
                            sb = int(S[kband])
                            catb = lp.tile([P, P], fp, tag="cat")
                            rin = msg[:, col * D : (col + sb) * D].rearrange(
                                "p (s c) -> p c s", c=D
                            )
                            col += sb
                            nc.vector.tensor_reduce(
                                out=catb[:, :D],
                                in_=rin,
                                axis=mybir.AxisListType.X,
                                op=mybir.AluOpType.add,
                            )
                            nc.scalar.copy(
                                out=catb[:, D:], in_=xd_ph[:, bloc * D : (bloc + 1) * D]
                            )
                            lps = psL.tile([P, P], fp, tag="lps")
                            nc.tensor.transpose(out=lps[:], in_=catb[:], identity=ident[:])
                            ltile = lp.tile([P, P], fp, tag="l")
                            nc.scalar.copy(out=ltile[:], in_=lps[:])
                            nc.tensor.matmul(
                                zp[:, bi * D : (bi + 1) * D], lhsT=ltile[:], rhs=wt_t[:],
                                start=True, stop=True,
                            )
                        # batched SELU epilogue on [P, g*D]
                        z = stg.tile([P, g * D], fp, tag="z1")
                        nc.vector.tensor_tensor(
                            out=z[:].rearrange("p (b c) -> p b c", c=D),
                            in0=zp[:].rearrange("p (b c) -> p b c", c=D),
                            in1=bsr_t[:].unsqueeze(1).broadcast_to([P, g, D]),
                            op=mybir.AluOpType.add,
                        )
                        en = stg.tile([P, g * D], fp, tag="z2")
                        nc.vector.tensor_scalar_min(out=en[:], in0=z[:], scalar1=0.0)
                        nc.scalar.activation(
                            out=en[:], in_=en[:], func=mybir.ActivationFunctionType.Exp
                        )
                        pos = stg.tile([P, g * D], fp, tag="z3")
                        nc.scalar.activation(
                            out=pos[:], in_=z[:], func=mybir.ActivationFunctionType.Relu,
                            scale=lam,
                        )
                        nc.vector.tensor_scalar(
                            out=en[:], in0=en[:], scalar1=la, scalar2=-la,
                            op0=mybir.AluOpType.mult, op1=mybir.AluOpType.add,
                        )
                        nc.vector.tensor_tensor(
                            out=pos[:], in0=pos[:], in1=en[:], op=mybir.AluOpType.add
                        )
                        nc.sync.dma_start(
                            out=outp[b0 * P : (b0 + g) * P, :].rearrange("(b p) c -> p b c", p=P),
                            in_=pos[:].rearrange("p (b c) -> p b c", c=D),
                        )
    nc.compile()
    return nc


class _Runner:
    """Reusable SPMD executor over axon PJRT (one jit, many runs)."""

    def __init__(self, nc, n_cores):
        import jax
        import concourse.mybir as mybir
        from jax.sharding import Mesh, PartitionSpec
        from jax.experimental.shard_map import shard_map
        from concourse.bass2jax import (
            _bass_exec_p,
            partition_id_tensor,
            install_neuronx_cc_hook,
        )

        install_neuronx_cc_hook()
        self.jax = jax
        self.n_cores = n_cores
        partition_name = nc.partition_id_tensor.name if nc.partition_id_tensor else None
        in_names, out_names, out_avals, zero_outs = [], [], [], []
        for alloc in nc.m.functions[0].allocations:
            if not isinstance(alloc, mybir.MemoryLocationSet):
                continue
            name = alloc.memorylocations[0].name
            if alloc.kind == "ExternalInput":
                if name != partition_name:
                    in_names.append(name)
            elif alloc.kind == "ExternalOutput":
                shape = tuple(alloc.tensor_shape)
                dtype = mybir.dt.np(alloc.dtype)
                out_avals.append(jax.core.ShapedArray(shape, dtype))
                out_names.append(name)
                zero_outs.append(np.zeros(shape, dtype))
        self.in_names, self.out_names = in_names, out_names
        self.out_avals, self.zero_outs = out_avals, zero_outs
        n_params, n_outs = len(in_names), len(out_avals)
        all_in = list(in_names) + list(out_names)
        if partition_name is not None:
            all_in.append(partition_name)

        def _body(*args):
            operands = list(args)
            if partition_name is not None:
                operands.append(partition_id_tensor())
            outs = _bass_exec_p.bind(
                *operands,
                out_avals=tuple(out_avals),
                in_names=tuple(all_in),
                out_names=tuple(out_names),
                lowering_input_output_aliases=(),
                sim_require_finite=True,
                sim_require_nnan=True,
                nc=nc,
            )
            return tuple(outs)

        devices = jax.devices()[:n_cores]
        assert len(devices) == n_cores, f"need {n_cores} cores, have {len(jax.devices())}"
        self.mesh = Mesh(np.asarray(devices), ("core",))
        in_specs = (PartitionSpec("core"),) * (n_params + n_outs)
        out_specs = (PartitionSpec("core"),) * n_outs
        self.jitted = jax.jit(
            shard_map(_body, mesh=self.mesh, in_specs=in_specs,
                      out_specs=out_specs, check_rep=False),
            donate_argnums=tuple(range(n_params, n_params + n_outs)),
            keep_unused=True,
        )
        self.n_params = n_params

    def put_inputs(self, in_maps):
        import jax
        from jax.sharding import PartitionSpec
        per_core = [[np.asarray(m[n]) for n in self.in_names] for m in in_maps]
        concat = [
            np.concatenate([per_core[c][i] for c in range(self.n_cores)], axis=0)
            for i in range(self.n_params)
        ]
        sh = jax.sharding.NamedSharding(self.mesh, PartitionSpec("core"))
        return [jax.device_put(a, sh) for a in concat]

    def run(self, dev_inputs):
        import jax
        from jax.sharding import PartitionSpec
        sh = jax.sharding.NamedSharding(self.mesh, PartitionSpec("core"))
        zeros = [
            jax.device_put(np.zeros((self.n_cores * z.shape[0], *z.shape[1:]), z.dtype), sh)
            for z in self.zero_outs
        ]
        outs = self.jitted(*dev_inputs, *zeros)
        jax.block_until_ready(outs)
        return outs

    def results(self, outs):
        return [
            {
                n: np.asarray(outs[i]).reshape(self.n_cores, *self.out_avals[i].shape)[c]
                for i, n in enumerate(self.out_names)
            }
            for c in range(self.n_cores)
        ]


def _get_compiled(S, phases, groups, tabsz, krep=1):
    key = (tuple(S.tolist()), tuple(groups), tuple(phases), tabsz, krep)
    if key not in _compiled:
        nc = _build_program(S, phases, groups, tabsz, krep=krep)
        _compiled[key] = _Runner(nc, NC)
    return _compiled[key]


def _prepare(features, W, bias, skip_weight, edge_weight, edge_src, edge_dst):
    deg, order, rank, S, phases, groups = _structure(edge_dst)
    (xtab, idx16_all, w_all, idxd16_all, tabsz, _, _, _) = _pack_host(
        features, edge_src, edge_dst, edge_weight, order, rank, S, phases, groups
    )
    wt_host = np.ascontiguousarray(np.vstack([W.T, W.T]))
    skr_host = np.ascontiguousarray(
        np.vstack([np.ones((D, D), np.float32), np.tile(skip_weight[None, :], (D, 1))])
    )
    bsr_host = np.ascontiguousarray(np.tile(bias[None, :], (P, 1)))
    ident_host = np.eye(P, dtype=np.float32)
    in_maps = [
        {
            "xtab": xtab[c],
            "idx16": idx16_all[c],
            "wgt": w_all[c],
            "idxd16": idxd16_all[c],
            "wt": wt_host,
            "skr": skr_host,
            "bsr": bsr_host,
            "ident": ident_host,
        }
        for c in range(NC)
    ]
    return order, S, phases, groups, tabsz, in_maps


def kernel(features, W, bias, skip_weight, edge_weight, edge_src, edge_dst):
    features = np.ascontiguousarray(np.asarray(features, dtype=np.float32))
    W = np.asarray(W, dtype=np.float32)
    bias = np.asarray(bias, dtype=np.float32)
    skip_weight = np.asarray(skip_weight, dtype=np.float32)
    edge_weight = np.asarray(edge_weight, dtype=np.float32)
    edge_src = np.asarray(edge_src, dtype=np.int32)
    edge_dst = np.asarray(edge_dst, dtype=np.int32)

    order, S, phases, groups, tabsz, in_maps = _prepare(
        features, W, bias, skip_weight, edge_weight, edge_src, edge_dst
    )
    runner = _get_compiled(S, phases, groups, tabsz, krep=1)
    dev_in = runner.put_inputs(in_maps)
    outs = runner.run(dev_in)
    res = runner.results(outs)

    nbands = len(S)
    out_full = np.empty((N, D), dtype=np.float32)
    for c in range(NC):
        rr = (np.arange(nbands)[:, None] * BAND + c * P + np.arange(P)[None, :]).ravel()
        valid = rr < N
        out_full[order[rr[valid]]] = res[c]["out"][valid]
    return out_full


# revision 15
# speedup vs baseline: 1.0328x; 1.0328x over previous
"""GCNConv on 8 Trainium2 NeuronCores (Bass/Tile, SPMD).

out = selu((X @ W.T) * skip_w  +  segsum_dst(w_e * X[src] @ W.T)  + bias)

Key algebraic trick: the linear transform commutes with the (linear) edge
aggregation, so raw features are aggregated first and W is applied once per
destination block:   agg_dst = (sum_e w_e * X[src_e]) @ W.T
No h = X@W.T round-trip through DRAM; the kernel gathers raw 256B feature
rows with the custom SWDGE dma_gather (InstDMAGatherAnt, int16 indices,
single_packet=False — the only gather path that is correct on HW for
many-rows-per-partition gathers).

Sharding: destination nodes are degree-sorted and dealt round-robin across
(core, partition); band k = 1024 consecutive degree-ranked nodes = one
128-row block per core, padded to S[k] = max degree in band (~3% padding).
Since dma_gather indices are int16, bands are split into a few phases, each
with a per-(core,phase) compacted feature table (< 32768 rows) built on the
host (structural gather of input rows only).

Per group of bands: one dma_gather (~10k rows) -> msg [128, sumS*64]; one
DVE broadcast multiply by edge weights; per band a strided DVE reduce ->
aggX, concat with gathered x_dst, one PE transpose -> L = [aggX.T ; xd.T],
one K=128 matmul against [W.T ; W.T*skip] into a PSUM stage; batched SELU
epilogue (Relu/Exp on ACT) and one output DMA per group. Host work is
structural only (sorting, packing, dedup, permutation, replication).
"""
import sys

sys.path.insert(0, "/opt/trn_rl_repo")
import numpy as np

N = 50000
E = 800000
D = 64
P = 128
NC = 8
BAND = P * NC  # 1024 nodes per band
SELU_LAMBDA = 1.0507009873554804934193349852946
SELU_ALPHA = 1.6732632423543772848170429916717
SMAX = 96      # max slots/partition per gather group (NIDX <= 12288, HW-validated)
GMAX = 8       # max bands per group (PSUM bank: 8*64 f32 = one bank)
TAB_LIMIT = 30000  # per-phase compact table safety bound (int16 indices)

_compiled = {}


def _structure(edge_dst):
    deg = np.bincount(edge_dst, minlength=N)
    order = np.argsort(-deg, kind="stable").astype(np.int64)  # rank -> node
    rank = np.empty(N, dtype=np.int64)
    rank[order] = np.arange(N)
    nbands = (N + BAND - 1) // BAND  # 49
    degs_sorted = deg[order]
    S = np.zeros(nbands, dtype=np.int64)
    band_edges = np.zeros(nbands, dtype=np.int64)
    for k in range(nbands):
        d = degs_sorted[k * BAND : (k + 1) * BAND]
        S[k] = max(1, int(d.max()) if len(d) else 1)
        band_edges[k] = int(d.sum())
    # split bands into phases with ~equal edge counts; grow count until the
    # expected per-(core,phase) distinct-source bound is comfortable
    for nph in range(2, 9):
        tgt = band_edges.sum() / nph
        cuts = [0]
        acc = 0
        for k in range(nbands):
            acc += band_edges[k]
            if acc >= tgt * len(cuts) and len(cuts) < nph:
                cuts.append(k + 1)
        if cuts[-1] != nbands:
            cuts.append(nbands)
        phases = [(cuts[i], cuts[i + 1]) for i in range(len(cuts) - 1)]
        worst = max(band_edges[a:b].sum() for a, b in phases) / NC
        # distinct sources upper estimate + dst nodes
        est = N * (1.0 - np.exp(-worst / N)) + 1024 * max(b - a for a, b in phases) / NC
        if est < TAB_LIMIT - 3000:
            break
    # groups within phases
    groups = []  # (b0, g, ssum, phase_id)
    for pi, (a, b) in enumerate(phases):
        b0 = a
        while b0 < b:
            g = 0
            ssum = 0
            while b0 + g < b and g < GMAX and (g == 0 or ssum + S[b0 + g] <= SMAX):
                ssum += S[b0 + g]
                g += 1
            groups.append((b0, g, int(ssum), pi))
            b0 += g
    return deg, order, rank, S, phases, groups


def _rep16(flat):
    """int16 flat index list (i = s*128+p order) -> [128, len/16] replicated tile."""
    b16 = flat.reshape(-1, 16).T  # [16, n/16]
    return np.tile(b16, (8, 1))   # [128, n/16]


def _pack_host(features, edge_src, edge_dst, edge_weight, order, rank, S, phases, groups):
    nbands = len(S)
    nph = len(phases)
    band_phase = np.zeros(nbands, dtype=np.int64)
    for pi, (a, b) in enumerate(phases):
        band_phase[a:b] = pi

    # edge -> (core, band, partition, slot)
    sort_idx = np.argsort(edge_dst, kind="stable")
    sdst = edge_dst[sort_idx]
    ssrc = edge_src[sort_idx]
    swgt = edge_weight[sort_idx].astype(np.float32)
    first = np.searchsorted(sdst, sdst, side="left")
    pos = np.arange(len(sdst)) - first
    rr = rank[sdst]
    kb = rr // BAND
    cc = (rr // P) % NC
    pp = rr % P
    ph = band_phase[kb]

    # per-(core,phase) id tables
    ids_list = [[None] * nph for _ in range(NC)]
    tabsz = 0
    for c in range(NC):
        for pi in range(nph):
            a, b = phases[pi]
            m = (cc == c) & (ph == pi)
            dsts = []
            for k in range(a, b):
                rrs = k * BAND + c * P + np.arange(P)
                dsts.append(order[np.minimum(rrs, N - 1)])
            ids = np.unique(np.concatenate([ssrc[m]] + dsts))
            assert len(ids) < 32768, f"phase table overflow: {len(ids)}"
            ids_list[c][pi] = ids
            tabsz = max(tabsz, len(ids))
    tabsz = (tabsz + 15) // 16 * 16

    xtab = np.zeros((NC, nph * tabsz, D), dtype=np.float32)
    for c in range(NC):
        for pi in range(nph):
            ids = ids_list[c][pi]
            xtab[c, pi * tabsz : pi * tabsz + len(ids)] = features[ids]

    # slot packing:
    # per group: slot arrays A[c][(p, scol)] -> compact idx / weight
    band_g = np.zeros(nbands, dtype=np.int64)      # group id of band
    band_coloff = np.zeros(nbands, dtype=np.int64)
    for gi, (b0, g, ssum, pi) in enumerate(groups):
        c0 = 0
        for k in range(b0, b0 + g):
            band_g[k] = gi
            band_coloff[k] = c0
            c0 += S[k]

    idx16_chunks = [[] for _ in range(NC)]
    w_chunks = [[] for _ in range(NC)]
    idx16_off = []   # per-group offset into the flat idx16 array (elements)
    w_off = []       # per-group offset into w array
    io_acc = 0
    wo_acc = 0
    for gi, (b0, g, ssum, pi) in enumerate(groups):
        idx16_off.append(io_acc)
        w_off.append(wo_acc)
        io_acc += P * 8 * ssum  # replicated [128, 8*ssum]
        wo_acc += P * ssum

    # build per-core compact idx (int16) and weights arrays group by group
    scol = band_coloff[kb] + pos  # slot column within group
    for c in range(NC):
        mc = cc == c
        gsel = band_g[kb[mc]]
        A_idx = [np.zeros((P, ssum), dtype=np.int16) for (_, _, ssum, _) in groups]
        A_w = [np.zeros((P, ssum), dtype=np.float32) for (_, _, ssum, _) in groups]
        # remap sources to compact per-phase ids
        for pi in range(nph):
            mm = mc & (ph == pi)
            comp = np.searchsorted(ids_list[c][pi], ssrc[mm]).astype(np.int16)
            gsel2 = band_g[kb[mm]]
            p2 = pp[mm]
            s2 = scol[mm]
            # scatter into per-group arrays
            for gi in np.unique(gsel2):
                m3 = gsel2 == gi
                A_idx[gi][p2[m3], s2[m3]] = comp[m3]
                A_w[gi][p2[m3], s2[m3]] = swgt[mm][m3]
        for gi, (b0, g, ssum, pi) in enumerate(groups):
            flat = A_idx[gi].T.ravel()  # i = s*128+p order
            idx16_chunks[c].append(_rep16(flat).ravel())
            w_chunks[c].append(A_w[gi].ravel())

    idx16_all = np.stack([np.concatenate(ch) for ch in idx16_chunks])
    w_all = np.stack([np.concatenate(ch) for ch in w_chunks])

    # dst-node gathers per phase (xd), compact int16: flat i = b_local*128+p
    xd_chunks = [[] for _ in range(NC)]
    xd_off = []
    xo_acc = 0
    for pi, (a, b) in enumerate(phases):
        xd_off.append(xo_acc)
        xo_acc += P * 8 * (b - a)
    for c in range(NC):
        for pi, (a, b) in enumerate(phases):
            nb_ph = b - a
            Axd = np.zeros((P, nb_ph), dtype=np.int16)
            for bl, k in enumerate(range(a, b)):
                rrs = k * BAND + c * P + np.arange(P)
                nodes = order[np.minimum(rrs, N - 1)]
                comp = np.searchsorted(ids_list[c][pi], nodes)
                # pad nodes (rr >= N) may not be in the table; clip (discarded)
                comp = np.clip(comp, 0, len(ids_list[c][pi]) - 1)
                Axd[:, bl] = comp.astype(np.int16)
            flat = Axd.T.ravel()
            xd_chunks[c].append(_rep16(flat).ravel())
    idxd16_all = np.stack([np.concatenate(ch) for ch in xd_chunks])

    return (
        xtab,
        idx16_all,
        w_all,
        idxd16_all,
        tabsz,
        idx16_off,
        w_off,
        xd_off,
    )


def _build_program(S, phases, groups, tabsz, krep=1):
    import concourse.bass as bass
    import concourse.bacc as bacc
    import concourse.mybir as mybir
    import concourse.tile as tile
    from concourse.library_config import mlp

    nbands = len(S)
    nph = len(phases)
    fp = mybir.dt.float32
    i16 = mybir.dt.int16

    toti16 = sum(P * 8 * ssum for (_, _, ssum, _) in groups)
    totw = sum(P * ssum for (_, _, ssum, _) in groups)
    totd16 = sum(P * 8 * (b - a) for (a, b) in phases)

    nc = bacc.Bacc(None, target_bir_lowering=False, debug=False)
    xtab = nc.declare_dram_parameter("xtab", [nph * tabsz, D], fp, isOutput=False)
    idxp = nc.declare_dram_parameter("idx16", [toti16], i16, isOutput=False)
    wp_d = nc.declare_dram_parameter("wgt", [totw], fp, isOutput=False)
    idxd = nc.declare_dram_parameter("idxd16", [totd16], i16, isOutput=False)
    wtp = nc.declare_dram_parameter("wt", [P, D], fp, isOutput=False)      # [W.T; W.T]
    skp = nc.declare_dram_parameter("skr", [P, D], fp, isOutput=False)     # [ones; skip]
    bsp = nc.declare_dram_parameter("bsr", [P, D], fp, isOutput=False)
    idp = nc.declare_dram_parameter("ident", [P, P], fp, isOutput=False)
    outp = nc.declare_dram_parameter("out", [nbands * P, D], fp, isOutput=True)

    lam = SELU_LAMBDA
    la = SELU_LAMBDA * SELU_ALPHA

    with tile.TileContext(nc) as tc:
        with (
            tc.tile_pool(name="const", bufs=1) as cst,
            tc.tile_pool(name="msgp", bufs=2) as msgp,
            tc.tile_pool(name="meta", bufs=3) as meta,
            tc.tile_pool(name="xdp", bufs=2) as xdp,
            tc.tile_pool(name="lp", bufs=3) as lp,
            tc.tile_pool(name="stg", bufs=2) as stg,
            tc.tile_pool(name="psL", bufs=2, space="PSUM") as psL,
            tc.tile_pool(name="psZ", bufs=2, space="PSUM") as psZ,
        ):
            nc.gpsimd.load_library(mlp)
            ident = cst.tile([P, P], fp)
            nc.sync.dma_start(out=ident[:], in_=idp[:])
            wt_t = cst.tile([P, D], fp)
            nc.sync.dma_start(out=wt_t[:], in_=wtp[:])
            skr_t = cst.tile([P, D], fp)
            nc.sync.dma_start(out=skr_t[:], in_=skp[:])
            nc.vector.tensor_tensor(out=wt_t[:], in0=wt_t[:], in1=skr_t[:], op=mybir.AluOpType.mult)
            bsr_t = cst.tile([P, D], fp)
            nc.sync.dma_start(out=bsr_t[:], in_=bsp[:])

            for _rep in range(krep):
                for pi, (a, b) in enumerate(phases):
                    nb_ph = b - a
                    tab_ap = xtab[pi * tabsz : (pi + 1) * tabsz, :]
                    # dst features for this phase's bands
                    xd_off_ap = sum(P * 8 * (bb - aa) for (aa, bb) in phases[:pi])
                    itd = meta.tile([P, 8 * nb_ph], i16, tag="itd")
                    nc.sync.dma_start(
                        out=itd[:],
                        in_=idxd[xd_off_ap : xd_off_ap + P * 8 * nb_ph].rearrange(
                            "(p s) -> p s", p=P
                        ),
                    )
                    xd_ph = xdp.tile([P, nb_ph * D], fp, tag="xd")
                    nc.gpsimd.dma_gather(
                        xd_ph[:].rearrange("p (s c) -> p s c", c=D),
                        tab_ap,
                        itd[:, :],
                        P * nb_ph,
                        P * nb_ph,
                        D,
                        single_packet=False,
                    )
                    for gi, (b0, g, ssum, gpi) in enumerate(groups):
                        if gpi != pi:
                            continue
                        ioff = sum(P * 8 * s2 for (_, _, s2, _) in groups[:gi])
                        woff = sum(P * s2 for (_, _, s2, _) in groups[:gi])
                        it = meta.tile([P, 8 * ssum], i16, tag="it")
                        nc.sync.dma_start(
                            out=it[:],
                            in_=idxp[ioff : ioff + P * 8 * ssum].rearrange("(p s) -> p s", p=P),
                        )
                        wt_w = meta.tile([P, ssum], fp, tag="w")
                        nc.sync.dma_start(
                            out=wt_w[:],
                            in_=wp_d[woff : woff + P * ssum].rearrange("(p s) -> p s", p=P),
                        )
                        msg = msgp.tile([P, ssum * D], fp, tag="msg")
                        nc.gpsimd.dma_gather(
                            msg[:].rearrange("p (s c) -> p s c", c=D),
                            tab_ap,
                            it[:, :],
                            P * ssum,
                            P * ssum,
                            D,
                            single_packet=False,
                        )
                        # weight multiply, whole group in one DVE op
                        m3 = msg[:].rearrange("p (s c) -> p s c", c=D)
                        nc.vector.tensor_tensor(
                            out=m3, in0=m3,
                            in1=wt_w[:].unsqueeze(2).broadcast_to([P, ssum, D]),
                            op=mybir.AluOpType.mult,
                        )
                        zp = psZ.tile([P, g * D], fp, tag="z")
                        col = 0
                        for bi in range(g):
                            kband = b0 + bi
                            bloc = kband - a  # band index within phase
                            sb = int(S[kband])
                            catb = lp.tile([P, P], fp, tag="cat")
                            rin = msg[:, col * D : (col + sb) * D].rearrange(
                                "p (s c) -> p c s", c=D
                            )
                            col += sb
                            nc.vector.tensor_reduce(
                                out=catb[:, :D],
                                in_=rin,
                                axis=mybir.AxisListType.X,
                                op=mybir.AluOpType.add,
                            )
                            nc.scalar.copy(
                                out=catb[:, D:], in_=xd_ph[:, bloc * D : (bloc + 1) * D]
                            )
                            lps = psL.tile([P, P], fp, tag="lps")
                            nc.tensor.transpose(out=lps[:], in_=catb[:], identity=ident[:])
                            ltile = lp.tile([P, P], fp, tag="l")
                            nc.scalar.copy(out=ltile[:], in_=lps[:])
                            nc.tensor.matmul(
                                zp[:, bi * D : (bi + 1) * D], lhsT=ltile[:], rhs=wt_t[:],
                                start=True, stop=True,
                            )
                        # batched SELU epilogue on [P, g*D]
                        z = stg.tile([P, g * D], fp, tag="z1")
                        nc.vector.tensor_tensor(
                            out=z[:].rearrange("p (b c) -> p b c", c=D),
                            in0=zp[:].rearrange("p (b c) -> p b c", c=D),
                            in1=bsr_t[:].unsqueeze(1).broadcast_to([P, g, D]),
                            op=mybir.AluOpType.add,
                        )
                        en = stg.tile([P, g * D], fp, tag="z2")
                        nc.vector.tensor_scalar_min(out=en[:], in0=z[:], scalar1=0.0)
                        nc.scalar.activation(
                            out=en[:], in_=en[:], func=mybir.ActivationFunctionType.Exp
                        )
                        pos = stg.tile([P, g * D], fp, tag="z3")
                        nc.scalar.activation(
                            out=pos[:], in_=z[:], func=mybir.ActivationFunctionType.Relu,
                            scale=lam,
                        )
                        nc.vector.tensor_scalar(
                            out=en[:], in0=en[:], scalar1=la, scalar2=-la,
                            op0=mybir.AluOpType.mult, op1=mybir.AluOpType.add,
                        )
                        nc.vector.tensor_tensor(
                            out=pos[:], in0=pos[:], in1=en[:], op=mybir.AluOpType.add
                        )
                        nc.sync.dma_start(
                            out=outp[b0 * P : (b0 + g) * P, :].rearrange("(b p) c -> p b c", p=P),
                            in_=pos[:].rearrange("p (b c) -> p b c", c=D),
                        )
    nc.compile()
    return nc


class _Runner:
    """Reusable SPMD executor over axon PJRT (one jit, many runs)."""

    def __init__(self, nc, n_cores):
        import jax
        import concourse.mybir as mybir
        from jax.sharding import Mesh, PartitionSpec
        from jax.experimental.shard_map import shard_map
        from concourse.bass2jax import (
            _bass_exec_p,
            partition_id_tensor,
            install_neuronx_cc_hook,
        )

        install_neuronx_cc_hook()
        self.jax = jax
        self.n_cores = n_cores
        partition_name = nc.partition_id_tensor.name if nc.partition_id_tensor else None
        in_names, out_names, out_avals, zero_outs = [], [], [], []
        for alloc in nc.m.functions[0].allocations:
            if not isinstance(alloc, mybir.MemoryLocationSet):
                continue
            name = alloc.memorylocations[0].name
            if alloc.kind == "ExternalInput":
                if name != partition_name:
                    in_names.append(name)
            elif alloc.kind == "ExternalOutput":
                shape = tuple(alloc.tensor_shape)
                dtype = mybir.dt.np(alloc.dtype)
                out_avals.append(jax.core.ShapedArray(shape, dtype))
                out_names.append(name)
                zero_outs.append(np.zeros(shape, dtype))
        self.in_names, self.out_names = in_names, out_names
        self.out_avals, self.zero_outs = out_avals, zero_outs
        n_params, n_outs = len(in_names), len(out_avals)
        all_in = list(in_names) + list(out_names)
        if partition_name is not None:
            all_in.append(partition_name)

        def _body(*args):
            operands = list(args)
            if partition_name is not None:
                operands.append(partition_id_tensor())
            outs = _bass_exec_p.bind(
                *operands,
                out_avals=tuple(out_avals),
                in_names=tuple(all_in),
                out_names=tuple(out_names),
                lowering_input_output_aliases=(),
                sim_require_finite=True,
                sim_require_nnan=True,
                nc=nc,
            )
            return tuple(outs)

        devices = jax.devices()[:n_cores]
        assert len(devices) == n_cores, f"need {n_cores} cores, have {len(jax.devices())}"
        self.mesh = Mesh(np.asarray(devices), ("core",))
        in_specs = (PartitionSpec("core"),) * (n_params + n_outs)
        out_specs = (PartitionSpec("core"),) * n_outs
        self.jitted = jax.jit(
            shard_map(_body, mesh=self.mesh, in_specs=in_specs,
                      out_specs=out_specs, check_rep=False),
            donate_argnums=tuple(range(n_params, n_params + n_outs)),
            keep_unused=True,
        )
        self.n_params = n_params

    def put_inputs(self, in_maps):
        import jax
        from jax.sharding import PartitionSpec
        per_core = [[np.asarray(m[n]) for n in self.in_names] for m in in_maps]
        concat = [
            np.concatenate([per_core[c][i] for c in range(self.n_cores)], axis=0)
            for i in range(self.n_params)
        ]
        sh = jax.sharding.NamedSharding(self.mesh, PartitionSpec("core"))
        return [jax.device_put(a, sh) for a in concat]

    def run(self, dev_inputs, donate_bufs=None):
        import jax
        from jax.sharding import PartitionSpec
        if donate_bufs is None:
            sh = jax.sharding.NamedSharding(self.mesh, PartitionSpec("core"))
            donate_bufs = [
                jax.device_put(np.zeros((self.n_cores * z.shape[0], *z.shape[1:]), z.dtype), sh)
                for z in self.zero_outs
            ]
        outs = self.jitted(*dev_inputs, *donate_bufs)
        jax.block_until_ready(outs)
        return outs

    def results(self, outs):
        return [
            {
                n: np.asarray(outs[i]).reshape(self.n_cores, *self.out_avals[i].shape)[c]
                for i, n in enumerate(self.out_names)
            }
            for c in range(self.n_cores)
        ]


def _get_compiled(S, phases, groups, tabsz, krep=1):
    key = (tuple(S.tolist()), tuple(groups), tuple(phases), tabsz, krep)
    if key not in _compiled:
        nc = _build_program(S, phases, groups, tabsz, krep=krep)
        _compiled[key] = _Runner(nc, NC)
    return _compiled[key]


def _prepare(features, W, bias, skip_weight, edge_weight, edge_src, edge_dst):
    deg, order, rank, S, phases, groups = _structure(edge_dst)
    (xtab, idx16_all, w_all, idxd16_all, tabsz, _, _, _) = _pack_host(
        features, edge_src, edge_dst, edge_weight, order, rank, S, phases, groups
    )
    wt_host = np.ascontiguousarray(np.vstack([W.T, W.T]))
    skr_host = np.ascontiguousarray(
        np.vstack([np.ones((D, D), np.float32), np.tile(skip_weight[None, :], (D, 1))])
    )
    bsr_host = np.ascontiguousarray(np.tile(bias[None, :], (P, 1)))
    ident_host = np.eye(P, dtype=np.float32)
    in_maps = [
        {
            "xtab": xtab[c],
            "idx16": idx16_all[c],
            "wgt": w_all[c],
            "idxd16": idxd16_all[c],
            "wt": wt_host,
            "skr": skr_host,
            "bsr": bsr_host,
            "ident": ident_host,
        }
        for c in range(NC)
    ]
    return order, S, phases, groups, tabsz, in_maps


def kernel(features, W, bias, skip_weight, edge_weight, edge_src, edge_dst):
    features = np.ascontiguousarray(np.asarray(features, dtype=np.float32))
    W = np.asarray(W, dtype=np.float32)
    bias = np.asarray(bias, dtype=np.float32)
    skip_weight = np.asarray(skip_weight, dtype=np.float32)
    edge_weight = np.asarray(edge_weight, dtype=np.float32)
    edge_src = np.asarray(edge_src, dtype=np.int32)
    edge_dst = np.asarray(edge_dst, dtype=np.int32)

    order, S, phases, groups, tabsz, in_maps = _prepare(
        features, W, bias, skip_weight, edge_weight, edge_src, edge_dst
    )
    runner = _get_compiled(S, phases, groups, tabsz, krep=1)
    dev_in = runner.put_inputs(in_maps)
    outs = runner.run(dev_in)
    res = runner.results(outs)

    nbands = len(S)
    out_full = np.empty((N, D), dtype=np.float32)
    for c in range(NC):
        rr = (np.arange(nbands)[:, None] * BAND + c * P + np.arange(P)[None, :]).ravel()
        valid = rr < N
        out_full[order[rr[valid]]] = res[c]["out"][valid]
    return out_full
